# revision 4
# baseline (speedup 1.0000x reference)
"""CenterLoss on 8 Trainium2 NeuronCores (Bass/Tile).

loss = clip(distmat * onehot(labels), 1e-12, 1e12).sum() / B
     = (sum_i clip(||x_i - c_{y_i}||^2, 1e-12, 1e12) + B*(C-1)*1e-12) / B

Data-parallel over the batch: each of the 8 cores gets 4096 rows of x and
labels plus the full (replicated) centers table.  The core processes 4
chunks of 1024 rows: one 1MB contiguous-ish DMA for x, one dma_gather
(custom GpSimd ucode, descriptor generation parallel across the 8 Q7
cores) for the label-selected center rows, then DVE subtract, ACT square,
and a DVE 3D-reduce to per-sample squared distances.  Distances are
clipped on-device; per-core partial sums are combined on the host (the
sanctioned scalar all-reduce).
"""

import numpy as np

BATCH, NUM_CLASSES, FEATURE_DIM = 32768, 1024, 256
N_CORES = 8
SHARD = BATCH // N_CORES  # 4096
P = 128
N_CHUNKS = 4
CHUNK = SHARD // N_CHUNKS  # 1024 rows
TPC = CHUNK // P  # tiles per chunk = 8
N_TILES = SHARD // P  # 32
CLAMP_MIN, CLAMP_MAX = 1e-12, 1e12

_CACHE: dict = {}


def _build_nc():
    import concourse.bacc as bacc
    import concourse.tile as tile
    from concourse import mybir

    f32 = mybir.dt.float32
    i16 = mybir.dt.int16

    nc = bacc.Bacc("TRN2", target_bir_lowering=False, debug=False)

    x_d = nc.dram_tensor("x", [SHARD, FEATURE_DIM], f32, kind="ExternalInput")
    # idx table for dma_gather: [128, SHARD//16] int16 where
    # idx[16c + q, s] = labels[chunk_base + s*16 + q] replicated for the 8
    # GpSimd cores (c in 0..7), chunks side by side (CHUNK//16 cols each).
    idx_d = nc.dram_tensor("labidx", [P, SHARD // 16], i16, kind="ExternalInput")
    cen_d = nc.dram_tensor(
        "centers", [NUM_CLASSES, FEATURE_DIM], f32, kind="ExternalInput"
    )
    out_d = nc.dram_tensor("out", [1, 1], f32, kind="ExternalOutput")

    ICOLS = CHUNK // 16  # idx columns per chunk

    with tile.TileContext(nc) as tc:
        with (
            tc.tile_pool(name="data", bufs=3) as data,
            tc.tile_pool(name="work", bufs=2) as work,
            tc.tile_pool(name="single", bufs=1) as single,
            tc.tile_pool(name="psum", bufs=1, space="PSUM") as psum,
        ):
            idx_all = single.tile([P, SHARD // 16], i16)
            nc.sync.dma_start(out=idx_all[:], in_=idx_d[:, :])

            acc = single.tile([P, N_TILES], f32)
            for c in range(N_CHUNKS):
                x_t = data.tile([P, CHUNK // P, FEATURE_DIM], f32, tag="x")
                nc.sync.dma_start(
                    out=x_t[:],
                    in_=x_d[c * CHUNK : (c + 1) * CHUNK, :].rearrange(
                        "(t p) e -> p t e", p=P
                    ),
                )
                g_t = data.tile([P, CHUNK // P, FEATURE_DIM], f32, tag="g")
                nc.gpsimd.dma_gather(
                    out_ap=g_t[:],
                    in_ap=cen_d[:, :],
                    idxs_ap=idx_all[:, c * ICOLS : (c + 1) * ICOLS],
                    num_idxs=CHUNK,
                    num_idxs_reg=CHUNK,
                    elem_size=FEATURE_DIM,
                )
                d_t = work.tile([P, CHUNK // P, FEATURE_DIM], f32, tag="d")
                nc.vector.tensor_tensor(
                    out=d_t[:], in0=x_t[:], in1=g_t[:], op=mybir.AluOpType.subtract
                )
                s_t = work.tile([P, CHUNK // P, FEATURE_DIM], f32, tag="s")
                nc.scalar.activation(
                    out=s_t[:],
                    in_=d_t[:],
                    func=mybir.ActivationFunctionType.Square,
                )
                nc.vector.reduce_sum(
                    out=acc[:, c * TPC : (c + 1) * TPC],
                    in_=s_t[:],
                    axis=mybir.AxisListType.X,
                )

            clipped = single.tile([P, N_TILES], f32)
            nc.vector.tensor_scalar(
                out=clipped[:],
                in0=acc[:],
                scalar1=float(CLAMP_MIN),
                scalar2=float(CLAMP_MAX),
                op0=mybir.AluOpType.max,
                op1=mybir.AluOpType.min,
            )
            rowsum = single.tile([P, 1], f32)
            nc.vector.reduce_sum(out=rowsum[:], in_=clipped[:], axis=mybir.AxisListType.X)

            ones = single.tile([P, 1], f32)
            nc.vector.memset(ones[:], 1.0)
            tot = psum.tile([1, 1], f32, space="PSUM")
            nc.tensor.matmul(out=tot[:], lhsT=rowsum[:], rhs=ones[:], start=True, stop=True)
            res = single.tile([1, 1], f32)
            nc.vector.tensor_copy(out=res[:], in_=tot[:])
            nc.sync.dma_start(out=out_d[:, :], in_=res[:])

    nc.finalize()
    return nc


def _make_idx_table(labels_shard: np.ndarray) -> np.ndarray:
    """[SHARD] int -> [128, SHARD//16] int16 dma_gather index table."""
    tab = np.empty((P, SHARD // 16), dtype=np.int16)
    icols = CHUNK // 16
    for c in range(N_CHUNKS):
        chunk = labels_shard[c * CHUNK : (c + 1) * CHUNK].astype(np.int16)
        block = chunk.reshape(icols, 16).T  # [16, icols]
        tab[:, c * icols : (c + 1) * icols] = np.tile(block, (P // 16, 1))
    return np.ascontiguousarray(tab)


def kernel(x: np.ndarray, centers: np.ndarray, labels: np.ndarray) -> np.ndarray:
    from concourse import bass_utils

    if "nc" not in _CACHE:
        _CACHE["nc"] = _build_nc()
    nc = _CACHE["nc"]

    x = np.ascontiguousarray(np.asarray(x, dtype=np.float32))
    centers = np.ascontiguousarray(np.asarray(centers, dtype=np.float32))
    lab = np.asarray(labels).astype(np.int64).reshape(N_CORES, SHARD)

    in_maps = []
    for c in range(N_CORES):
        # per-chunk [p, t] layout matches x view "(t p) e -> p t e"
        in_maps.append({
            "x": np.ascontiguousarray(x.reshape(N_CORES, SHARD, FEATURE_DIM)[c]),
            "labidx": _make_idx_table(lab[c]),
            "centers": centers,
        })

    rr = bass_utils.run_bass_kernel_spmd(nc, in_maps, list(range(N_CORES)))
    _CACHE["last_results"] = rr

    total = sum(float(r["out"][0, 0]) for r in rr.results)
    loss = (total + BATCH * (NUM_CLASSES - 1) * CLAMP_MIN) / BATCH
    return np.asarray(loss, dtype=np.float32)


# revision 5
# speedup vs baseline: 1.1319x; 1.1319x over previous
"""CenterLoss on 8 Trainium2 NeuronCores (Bass/Tile).

loss = clip(distmat * onehot(labels), 1e-12, 1e12).sum() / B
     = (sum_i clip(||x_i - c_{y_i}||^2, 1e-12, 1e12) + B*(C-1)*1e-12) / B

Data-parallel over the batch: each of the 8 cores gets 4096 rows of x and
labels plus the full (replicated) centers table.  x is loaded in 4 big
DMAs; the label-selected center rows are fetched 128 at a time with
indirect DMAs (GpSimd SWDGE); per 128-row tile the vector engine computes
x-c and the scalar engine squares with a fused per-sample row-sum.
Distances are clipped on-device; per-core partial sums are combined on
the host (the sanctioned scalar all-reduce).
"""

import numpy as np

BATCH, NUM_CLASSES, FEATURE_DIM = 32768, 1024, 256
N_CORES = 8
SHARD = BATCH // N_CORES  # 4096
P = 128
N_CHUNKS = 4
CHUNK = SHARD // N_CHUNKS  # 1024 rows
TPC = CHUNK // P  # tiles per chunk = 8
N_TILES = SHARD // P  # 32
CLAMP_MIN, CLAMP_MAX = 1e-12, 1e12

_CACHE: dict = {}


def _build_nc():
    import concourse.bacc as bacc
    import concourse.bass as bass
    import concourse.tile as tile
    from concourse import mybir

    f32 = mybir.dt.float32
    i32 = mybir.dt.int32

    nc = bacc.Bacc("TRN2", target_bir_lowering=False, debug=False)

    x_d = nc.dram_tensor("x", [SHARD, FEATURE_DIM], f32, kind="ExternalInput")
    # labels pre-transposed on host to [P, N_TILES]: lab[p, t] = labels[t*P + p]
    lab_d = nc.dram_tensor("labels", [P, N_TILES], i32, kind="ExternalInput")
    cen_d = nc.dram_tensor(
        "centers", [NUM_CLASSES, FEATURE_DIM], f32, kind="ExternalInput"
    )
    out_d = nc.dram_tensor("out", [1, 1], f32, kind="ExternalOutput")

    with tile.TileContext(nc) as tc:
        with (
            tc.tile_pool(name="data", bufs=3) as data,
            tc.tile_pool(name="gbuf", bufs=6) as gbuf,
            tc.tile_pool(name="work", bufs=4) as work,
            tc.tile_pool(name="single", bufs=1) as single,
            tc.tile_pool(name="psum", bufs=1, space="PSUM") as psum,
        ):
            lab_all = single.tile([P, N_TILES], i32)
            nc.sync.dma_start(out=lab_all[:], in_=lab_d[:, :])

            acc = single.tile([P, N_TILES], f32)
            for c in range(N_CHUNKS):
                x_t = data.tile([P, TPC, FEATURE_DIM], f32, tag="x")
                nc.sync.dma_start(
                    out=x_t[:],
                    in_=x_d[c * CHUNK : (c + 1) * CHUNK, :].rearrange(
                        "(t p) e -> p t e", p=P
                    ),
                )
                for j in range(TPC):
                    t = c * TPC + j
                    g_t = gbuf.tile([P, FEATURE_DIM], f32, tag="g")
                    nc.gpsimd.indirect_dma_start(
                        out=g_t[:],
                        out_offset=None,
                        in_=cen_d[:, :],
                        in_offset=bass.IndirectOffsetOnAxis(
                            ap=lab_all[:, t : t + 1], axis=0
                        ),
                    )
                    d_t = work.tile([P, FEATURE_DIM], f32, tag="d")
                    nc.vector.tensor_tensor(
                        out=d_t[:],
                        in0=x_t[:, j, :],
                        in1=g_t[:],
                        op=mybir.AluOpType.subtract,
                    )
                    s_t = work.tile([P, FEATURE_DIM], f32, tag="s")
                    nc.scalar.activation(
                        out=s_t[:],
                        in_=d_t[:],
                        func=mybir.ActivationFunctionType.Square,
                        accum_out=acc[:, t : t + 1],
                    )

            clipped = single.tile([P, N_TILES], f32)
            nc.vector.tensor_scalar(
                out=clipped[:],
                in0=acc[:],
                scalar1=float(CLAMP_MIN),
                scalar2=float(CLAMP_MAX),
                op0=mybir.AluOpType.max,
                op1=mybir.AluOpType.min,
            )
            rowsum = single.tile([P, 1], f32)
            nc.vector.reduce_sum(out=rowsum[:], in_=clipped[:], axis=mybir.AxisListType.X)

            ones = single.tile([P, 1], f32)
            nc.vector.memset(ones[:], 1.0)
            tot = psum.tile([1, 1], f32, space="PSUM")
            nc.tensor.matmul(out=tot[:], lhsT=rowsum[:], rhs=ones[:], start=True, stop=True)
            res = single.tile([1, 1], f32)
            nc.vector.tensor_copy(out=res[:], in_=tot[:])
            nc.sync.dma_start(out=out_d[:, :], in_=res[:])

    nc.finalize()
    return nc


def kernel(x: np.ndarray, centers: np.ndarray, labels: np.ndarray) -> np.ndarray:
    from concourse import bass_utils

    if "nc" not in _CACHE:
        _CACHE["nc"] = _build_nc()
    nc = _CACHE["nc"]

    x = np.ascontiguousarray(np.asarray(x, dtype=np.float32))
    centers = np.ascontiguousarray(np.asarray(centers, dtype=np.float32))
    lab = np.asarray(labels).astype(np.int32).reshape(N_CORES, N_TILES, P)

    xs = x.reshape(N_CORES, SHARD, FEATURE_DIM)
    in_maps = [
        {
            "x": np.ascontiguousarray(xs[c]),
            "labels": np.ascontiguousarray(lab[c].T),  # [P, N_TILES]
            "centers": centers,
        }
        for c in range(N_CORES)
    ]

    rr = bass_utils.run_bass_kernel_spmd(nc, in_maps, list(range(N_CORES)))
    _CACHE["last_results"] = rr

    total = sum(float(r["out"][0, 0]) for r in rr.results)
    loss = (total + BATCH * (NUM_CLASSES - 1) * CLAMP_MIN) / BATCH
    return np.asarray(loss, dtype=np.float32)
